# revision 36
# baseline (speedup 1.0000x reference)
"""DFlashAttention (paged KV cache decode-attention block) on 8 Trainium2
NeuronCores.

Sharding: tensor-parallel over heads. Each core owns HQ/8 = 4 query heads and
HK/8 = 1 KV head (GQA group). Wq/Wk/Wv row-sharded, Wo column-sharded; each
core produces a partial output [B*S, HID] (stored bf16) which is summed on
the host.

v3 schedule (bf16 data path, PE-saturating):
  - All inputs are pre-laid-out on the host as contiguous [128, N] SBUF
    images so every DMA moves multi-KB per-partition lines at full HBM
    bandwidth.
  - One deadline-ordered DMA stream on the SP queue feeds everything into
    persistent tiles (no write-after-read hazards, so the queue never
    stalls): consts, interleaved x/Wq pieces, sin-cos, Wk, Wv, then the
    per-batch KV-cache parts and Wo ordered by first use.
  - Phase A: Q projection (chunk-major, heads inner) -> K stream -> V
    stream.  The q/k norm+rope chains (ACT/DVE/swap) and their sum matmuls
    are interleaved under the K/V streams, so attention starts with no
    bridge gap.  Sqrt/Exp table loads are prefetched into ACT-idle windows.
  - Scores computed transposed [l_chunk, (h,s)] so PV consumes probs
    directly; softmax sums via all-ones stationary matmuls fed by an eager
    8-deep DVE pre-reduction forest.  Cache-tail masking via per-partition
    Exp bias.
  - Output projection for batch b is emitted inside batch b+1's attention
    so its matmuls fill PE gaps; batches processed in descending length.
"""

import sys

sys.path.insert(0, "/opt/trn_rl_repo")

import numpy as np

B, S, HID = 4, 128, 4096
D, HQ, HK = 128, 32, 8
PAGES, PSIZE, NPP = 64, 256, 16
THETA = 10000.0
EPS = 1e-6
N_CORES = 8
HQC = HQ // N_CORES  # 4 query heads per core
EC = HQC * D         # 512 output-proj contraction per core
BS = B * S           # 512 tokens
NDCH = HID // 128    # 32 contraction chunks for projections

_CACHE = {}


def _img(mT):
    """[HID-like rows, C cols] -> contiguous SBUF image [128, (chunk, C)]."""
    r, c = mT.shape
    return np.ascontiguousarray(
        mT.reshape(r // 128, 128, c).transpose(1, 0, 2).reshape(128, -1))


def _prep_host(x, Wq, Wk, Wv, Wo, q_norm_w, k_norm_w, k_cache, v_cache,
               block_table, cache_seqlens):
    import ml_dtypes
    BF = ml_dtypes.bfloat16
    f32 = np.float32

    xT = np.asarray(x, f32).reshape(BS, HID).T          # [HID, BS]
    xS = _img(xT).astype(BF)                            # [128, 32*512]

    lens = [int(v) for v in np.asarray(cache_seqlens)]
    pads = [(l + 127) // 128 * 128 for l in lens]
    offs = [0] * B
    for b in range(1, B):
        offs[b] = offs[b - 1] + pads[b - 1]
    total = max(sum(pads), 128)

    bt = np.asarray(block_table)
    kg = np.asarray(k_cache, f32)[bt].reshape(B, NPP * PSIZE, HK, D)
    vg = np.asarray(v_cache, f32)[bt].reshape(B, NPP * PSIZE, HK, D)

    # RoPE sin/cos evaluated on host in f64 on the reference's fp32 freqs.
    pos = np.asarray(cache_seqlens, np.float64)[:, None] + np.arange(S)[None, :]
    inv = 1.0 / (THETA ** (np.arange(0, D, 2, dtype=np.float64) / D))
    freqs32 = (pos.astype(f32)[:, :, None]
               * inv.astype(f32)[None, None, :]).astype(f32)
    fr = np.float64(freqs32)
    sin_h = np.sin(fr).reshape(BS, 64).T            # [64, BS]
    cos_h = np.cos(fr).reshape(BS, 64).T
    # sign-folded: rope combine becomes a single add over all partitions
    sin2 = np.concatenate([-sin_h, sin_h], 0)       # [128, BS]
    cos2 = np.concatenate([cos_h, cos_h], 0)
    wq_ = np.asarray(q_norm_w, f32).reshape(D)
    wk_ = np.asarray(k_norm_w, f32).reshape(D)
    # norm weights premultiplied into the tables (tensor_tensor on DVE gets
    # the 2x bf16 mode; scalar_tensor_tensor does not)
    sinq = sin2 * np.roll(wq_, 64)[:, None]
    cosq = cos2 * wq_[:, None]
    sink = sin2 * np.roll(wk_, 64)[:, None]
    cosk = cos2 * wk_[:, None]
    # q tables doubled along columns so one op covers a 2-head pair tile
    sct = np.ascontiguousarray(np.concatenate(
        [sinq, sinq, cosq, cosq, sink, cosk], 1)).astype(BF)  # [128, 6*BS]

    # f32 per-partition scalars: epsq epsk + 4 mask-bias cols
    cols = [np.full(D, D * EPS, f32), np.full(D, EPS, f32)]
    for b in range(B):
        tail = lens[b] - (pads[b] // 128 - 1) * 128 if pads[b] > 0 else 128
        mb = np.zeros(D, f32)
        mb[tail:] = -1e30
        cols.append(mb)
    cf = np.ascontiguousarray(np.stack(cols, 1))    # [128, 6]

    # bf16 consts: all-ones stationary for partition-broadcast sums
    cb = np.ones((128, 128), f32).astype(BF)

    Wq_ = np.asarray(Wq, f32)
    Wk_ = np.asarray(Wk, f32)
    Wv_ = np.asarray(Wv, f32)
    Wo_ = np.asarray(Wo, f32)

    in_maps = []
    for c in range(N_CORES):
        # Wq image, head-pair-major: [128, (pair, chunk, head-in-pair, 128)]
        # so each pair's stream is a contiguous 2 MB block
        wqI = _img(Wq_[c * EC:(c + 1) * EC, :].T)   # [128, (chunk, h, 128)]
        wqI = wqI.reshape(128, NDCH, 2, 2 * D)
        wqS = np.ascontiguousarray(
            wqI.transpose(0, 2, 1, 3).reshape(128, NDCH * EC)).astype(BF)
        wkS = _img(Wk_[c * D:(c + 1) * D, :].T).astype(BF)
        wvS = _img(Wv_[c * D:(c + 1) * D, :].T).astype(BF)
        # Wo image, half-major: [128, (half, h, 1024)]
        woT = np.ascontiguousarray(Wo_[:, c * EC:(c + 1) * EC].T)  # [EC, HID]
        woS = np.ascontiguousarray(
            woT.reshape(HQC, 128, 4, 1024).transpose(1, 2, 0, 3)
            .reshape(128, 4 * HID)).astype(BF)
        kT = np.zeros((128, total), f32)
        vCf = np.zeros((total, 128), f32)
        for b in range(B):
            nb, ob = lens[b], offs[b]
            if nb > 0:
                kT[:, ob:ob + nb] = kg[b, :nb, c, :].T
                vCf[ob:ob + nb, :] = vg[b, :nb, c, :]
        vP = np.ascontiguousarray(
            vCf.reshape(total // 128, 128, 128).transpose(1, 0, 2)
            .reshape(128, total))
        in_maps.append(dict(
            xS=xS, wqS=wqS, wkS=wkS, wvS=wvS, woS=woS,
            kT=np.ascontiguousarray(kT).astype(BF), vC=vP.astype(BF),
            sct=sct, cf=cf, cb=cb,
        ))
    return in_maps, lens, pads, offs, total


def _build_nc(lens, pads, offs, total, reps=1):
    import concourse.mybir as mybir
    import concourse.tile as tile
    from concourse import bacc

    F32 = mybir.dt.float32
    BF16 = mybir.dt.bfloat16
    AF = mybir.ActivationFunctionType
    OP = mybir.AluOpType

    nc = bacc.Bacc("TRN2", target_bir_lowering=False, debug=False,
                   num_devices=N_CORES)

    xS_d = nc.dram_tensor("xS", [128, NDCH * BS], BF16, kind="ExternalInput")
    wqS_d = nc.dram_tensor("wqS", [128, NDCH * EC], BF16, kind="ExternalInput")
    wkS_d = nc.dram_tensor("wkS", [128, NDCH * D], BF16, kind="ExternalInput")
    wvS_d = nc.dram_tensor("wvS", [128, NDCH * D], BF16, kind="ExternalInput")
    woS_d = nc.dram_tensor("woS", [128, 4 * HID], BF16, kind="ExternalInput")
    kT_d = nc.dram_tensor("kT", [128, total], BF16, kind="ExternalInput")
    vC_d = nc.dram_tensor("vC", [128, total], BF16, kind="ExternalInput")
    sct_d = nc.dram_tensor("sct", [128, 6 * BS], BF16, kind="ExternalInput")
    cf_d = nc.dram_tensor("cf", [128, 6], F32, kind="ExternalInput")
    cb_d = nc.dram_tensor("cb", [128, 128], BF16, kind="ExternalInput")
    out_d = nc.dram_tensor("out", [BS, HID], BF16, kind="ExternalOutput")

    nch = [pads[b] // 128 for b in range(B)]
    desc_ = sorted(range(B), key=lambda b: -nch[b])
    border = [desc_[-1]] + desc_[:-1]

    with tile.TileContext(nc) as tc:
        with tc.tile_pool(name="const", bufs=1) as cpool, \
             tc.tile_pool(name="pers", bufs=1) as pers, \
             tc.tile_pool(name="wqp", bufs=4) as wqp, \
             tc.tile_pool(name="sqp", bufs=2) as sqp, \
             tc.tile_pool(name="srp", bufs=1) as srp, \
             tc.tile_pool(name="rsp", bufs=2) as rsp, \
             tc.tile_pool(name="tp", bufs=2) as tp, \
             tc.tile_pool(name="twp", bufs=2) as twp, \
             tc.tile_pool(name="mp", bufs=3) as mp, \
             tc.tile_pool(name="probp", bufs=4) as probp, \
             tc.tile_pool(name="recp", bufs=1) as recp, \
             tc.tile_pool(name="pp2", bufs=5) as pp2, \
             tc.tile_pool(name="odp", bufs=3) as odp, \
             tc.tile_pool(name="psS", bufs=3, space="PSUM") as psS, \
             tc.tile_pool(name="psO", bufs=1, space="PSUM") as psO:

            def _outproj_half(b, o_sb_t, wot_t, half, act_copy=False):
                    ps_out = psS.tile([128, 1024], F32, tag="ps",
                                      name=f"po{b}_{half}")
                    for h in range(HQC):
                        for hc in range(2):
                            nc.tensor.matmul(
                                ps_out[:, hc * 512:(hc + 1) * 512],
                                o_sb_t[:, b * 512 + h * D:
                                       b * 512 + (h + 1) * D],
                                wot_t[:, half * HID + h * 1024 + hc * 512:
                                      half * HID + h * 1024 + (hc + 1) * 512],
                                start=(h == 0), stop=(h == HQC - 1))
                    od = odp.tile([128, 1024], BF16, tag="od")
                    if act_copy:
                        # copies split across ACT+DVE, one DMA per half
                        nc.vector.tensor_copy(od[:, 0:512], ps_out[:, 0:512])
                        nc.scalar.activation(
                            od[:, 512:1024], ps_out[:, 512:1024],
                            mybir.ActivationFunctionType.Copy)
                        nc.sync.dma_start(
                            out=out_d[b * S:(b + 1) * S,
                                      half * 1024:(half + 1) * 1024],
                            in_=od[:, :])
                    else:
                        nc.vector.tensor_copy(od[:, :], ps_out[:, :])
                        nc.sync.dma_start(
                            out=out_d[b * S:(b + 1) * S,
                                      half * 1024:(half + 1) * 1024],
                            in_=od[:, :])

            def _outproj(b, o_sb_t, wot_t, act_copy=False):
                for half in range(4):
                    _outproj_half(b, o_sb_t, wot_t, half, act_copy=act_copy)

            def body(_it, first=True):
                # ---- persistent tiles ----
                cbt = cpool.tile([128, 128], BF16, tag="cb")
                nc.gpsimd.memset(cbt[:, :], 1.0)
                cft = cpool.tile([128, 6], F32, tag="cf")
                kTt = pers.tile([128, total], BF16, tag="kT")
                vCt = pers.tile([128, total], BF16, tag="vC")
                sct = cpool.tile([128, 6 * BS], BF16, tag="sct")
                xs = pers.tile([128, NDCH * BS], BF16, tag="xs")
                wkt = pers.tile([128, NDCH * D], BF16, tag="wk")
                wvt = pers.tile([128, NDCH * D], BF16, tag="wv")
                wot = pers.tile([128, 4 * HID], BF16, tag="wo")
                q_sb = pers.tile([128, HQC * BS], BF16, tag="q_sb")
                k_sb = pers.tile([128, BS], BF16, tag="k_sb")
                v_sb = pers.tile([128, BS], BF16, tag="v_sb")
                vt = pers.tile([128, BS], BF16, tag="vt")
                o_sb = pers.tile([128, B * 512], BF16, tag="o_sb")

                # ---- start of the ordered DMA stream (SP queue; strict
                # FIFO into persistent tiles = deadline-ordered transfers,
                # the queue never blocks on tile reuse) ----
                allones = cbt[:, 0:128]
                epsq, epsk = cft[:, 0:1], cft[:, 1:2]
                maskb = [cft[:, 2 + b:3 + b] for b in range(B)]
                sinq = sct[:, 0:2 * BS]
                cosq = sct[:, 2 * BS:4 * BS]
                sink = sct[:, 4 * BS:5 * BS]
                cosk = sct[:, 5 * BS:6 * BS]
                atl1 = cpool.tile([128, 1], F32, tag="atl1")
                atl2 = cpool.tile([128, 1], F32, tag="atl2")

                # ---- phase A part 1: Q projection in two head-pair passes
                # (pair 0's norm chain runs under pair 1's matmul stream) ----
                ps_q01 = psS.tile([128, 1024], F32, tag="ps", name="ps_q01")
                ps_q23 = psS.tile([128, 1024], F32, tag="ps", name="ps_q23")
                ps_qp = [ps_q01, ps_q23]
                sqs, tsbs, tsws, rqs = [], [], [], []
                sss = [psO.tile([128, 1024], F32, tag="po", name=f"ss{i}")
                       for i in range(2)]

                def qchain_pre(i):
                    # ACT part of pair i's norm chain (after ps stops)
                    sq = sqp.tile([128, 1024], BF16, tag="sq", name=f"sq{i}")
                    nc.scalar.activation(sq[:, :], ps_qp[i][:, :], AF.Square)
                    sqs.append(sq)

                def qsum(i):
                    # PE sum + ACT rstd for pair i (emit inside a PE stream)
                    for hc in range(2):
                        nc.tensor.matmul(
                            sss[i][:, hc * 512:(hc + 1) * 512], allones,
                            sqs[i][:, hc * 512:(hc + 1) * 512],
                            start=True, stop=True)
                    sr = srp.tile([128, 1024], BF16, tag="sr", name=f"sr{i}")
                    nc.scalar.activation(sr[:, :], sss[i][:, :], AF.Sqrt,
                                         bias=epsq, scale=1.0)
                    rq = rsp.tile([128, 1024], BF16, tag="rs", name=f"rq{i}")
                    with nc.allow_low_precision(reason="bf16 rstd"):
                        nc.vector.reciprocal(rq[:, :], sr[:, :])
                    rqs.append(rq)

                def qrope(i):
                    # DVE part: rope mults straight off PSUM (the rotate-half
                    # reads PSUM at a crossed base partition, which the
                    # verifier allows for non-SBUF inputs) + rstd scale
                    m1 = mp.tile([128, 1024], BF16, tag="m", name=f"m1_{i}")
                    nc.vector.tensor_mul(m1[:, :], ps_qp[i][:, :],
                                         cosq[:, 0:1024])
                    m2 = mp.tile([128, 1024], BF16, tag="m", name=f"m2_{i}")
                    nc.vector.tensor_mul(m2[0:64, :], ps_qp[i][64:128, :],
                                         sinq[0:64, 0:1024])
                    nc.vector.tensor_mul(m2[64:128, :], ps_qp[i][0:64, :],
                                         sinq[64:128, 0:1024])
                    rt = mp.tile([128, 1024], BF16, tag="m", name=f"rt{i}")
                    nc.vector.tensor_add(rt[:, :], m1[:, :], m2[:, :])
                    nc.vector.tensor_mul(q_sb[:, i * 1024:(i + 1) * 1024],
                                         rt[:, :], rqs[i][:, :])

                GRP = 4
                NP = NDCH // GRP

                def qpass(pair, per_g=None):
                    base = NDCH * 2 * D if pair else 0
                    for g in range(NP):
                        wq = wqp.tile([128, GRP * 2 * D], BF16, tag="wq")
                        if pair == 0 and g == 0:
                            nc.sync.dma_start(out=xs[:, 0:2 * BS],
                                              in_=xS_d[:, 0:2 * BS])
                            nc.sync.dma_start(out=wq[:, :],
                                              in_=wqS_d[:, 0:1024])
                            nc.sync.dma_start(out=xs[:, 2 * BS:GRP * BS],
                                              in_=xS_d[:, 2 * BS:GRP * BS])
                            nc.sync.dma_start(out=cft[:, :], in_=cf_d[:, :])
                        else:
                            nc.sync.dma_start(
                                out=wq[:, :],
                                in_=wqS_d[:, base + g * 1024:
                                          base + (g + 1) * 1024])
                            if pair == 0:
                                nc.sync.dma_start(
                                    out=xs[:, g * GRP * BS:(g + 1) * GRP * BS],
                                    in_=xS_d[:, g * GRP * BS:
                                             (g + 1) * GRP * BS])
                        if pair == 0 and g == 3:
                            # prefetch the Sqrt act table while ACT is idle
                            nc.scalar.activation(atl1[:, :], cft[:, 0:1],
                                                 AF.Sqrt)
                        if per_g:
                            per_g(g)
                        for j in range(GRP):
                            dch = g * GRP + j
                            st = dch == 0
                            sp = dch == NDCH - 1
                            xa = xs[:, dch * BS:(dch + 1) * BS]
                            for hp in range(2):
                                nc.tensor.matmul(
                                    ps_qp[pair][:, hp * 512:(hp + 1) * 512],
                                    wq[:, j * 2 * D + hp * D:
                                       j * 2 * D + (hp + 1) * D],
                                    xa, start=st, stop=sp)
                    qchain_pre(pair)

                # ---- pair-0 Q pass; then the K stream absorbs the x-DMA
                # pacing while pair-0's norm chain runs ----
                qpass(0)
                half_kc = NDCH * D // 2
                for i in range(2):
                    nc.sync.dma_start(
                        out=wkt[:, i * half_kc:(i + 1) * half_kc],
                        in_=wkS_d[:, i * half_kc:(i + 1) * half_kc])
                nc.sync.dma_start(out=sct[:, :], in_=sct_d[:, :])
                ps_k = psS.tile([128, 512], F32, tag="ps", name="ps_k")
                for j in range(NDCH):
                    nc.tensor.matmul(ps_k[:, :], wkt[:, j * D:(j + 1) * D],
                                     xs[:, j * BS:(j + 1) * BS],
                                     start=(j == 0), stop=(j == NDCH - 1))
                    if j == 6:
                        qsum(0)
                qrope(0)
                sqk = sqp.tile([128, 512], BF16, tag="sq", name="sqk")
                nc.scalar.activation(sqk[:, :], ps_k[:, :], AF.Square)

                # ---- pair-1 Q pass; k-norm sums + Exp-table prefetch under
                # it ----
                ssk = psO.tile([128, 512], F32, tag="po", name="ssk")
                rk = rsp.tile([128, 512], BF16, tag="rs", name="rk")

                def p1_per_g(g):
                    if g == 2:
                        nc.tensor.matmul(ssk[:, :], allones, sqk[:, :],
                                         start=True, stop=True)
                        srk = srp.tile([128, 512], BF16, tag="sr", name="srk")
                        nc.scalar.activation(srk[:, :], ssk[:, :], AF.Sqrt,
                                             bias=epsk, scale=1.0 / D)
                        # prefetch Exp table: all Sqrt uses are done
                        nc.scalar.activation(atl2[:, :], cft[:, 0:1], AF.Exp)
                        with nc.allow_low_precision(reason="bf16 rstd"):
                            nc.vector.reciprocal(rk[:, :], srk[:, :])

                qpass(1, per_g=p1_per_g)

                # rest of the input stream, in deadline order
                for i in range(2):
                    nc.sync.dma_start(
                        out=wvt[:, i * half_kc:(i + 1) * half_kc],
                        in_=wvS_d[:, i * half_kc:(i + 1) * half_kc])

                def ld_kT(b):
                    nc.sync.dma_start(
                        out=kTt[:, offs[b]:offs[b] + pads[b]],
                        in_=kT_d[:, offs[b]:offs[b] + pads[b]])

                def ld_vC(b):
                    nc.sync.dma_start(
                        out=vCt[:, offs[b]:offs[b] + pads[b]],
                        in_=vC_d[:, offs[b]:offs[b] + pads[b]])

                def ld_wo(i):
                    nc.sync.dma_start(
                        out=wot[:, i * HID:(i + 1) * HID],
                        in_=woS_d[:, i * HID:(i + 1) * HID])

                ld_kT(border[0])
                ld_vC(border[0])
                ld_kT(border[1])
                ld_vC(border[1])
                for i in range(4):
                    ld_wo(i)

                # ---- V stream; pair-1 q-chain + k rope run under it ----
                ps_v = psS.tile([128, 512], F32, tag="ps", name="ps_v")
                for j in range(NDCH):
                    nc.tensor.matmul(ps_v[:, :], wvt[:, j * D:(j + 1) * D],
                                     xs[:, j * BS:(j + 1) * BS],
                                     start=(j == 0), stop=(j == NDCH - 1))
                    if j == 2:
                        qsum(1)
                qrope(1)
                # fresh-V copies + transposes per batch in service order so
                # the first batch's fresh chunk is ready earliest
                for b in border:
                    nc.vector.tensor_copy(v_sb[:, b * S:(b + 1) * S],
                                          ps_v[:, b * S:(b + 1) * S])
                    nc.sync.dma_start_transpose(
                        vt[:, b * S:(b + 1) * S], v_sb[:, b * S:(b + 1) * S])
                ld_kT(border[2])
                ld_vC(border[2])
                ld_kT(border[3])
                ld_vC(border[3])
                m1k = mp.tile([128, 512], BF16, tag="m", name="m1k")
                nc.vector.tensor_mul(m1k[:, :], ps_k[:, :], cosk[:, :])
                m2k = mp.tile([128, 512], BF16, tag="m", name="m2k")
                nc.vector.tensor_mul(m2k[0:64, :], ps_k[64:128, :],
                                     sink[0:64, :])
                nc.vector.tensor_mul(m2k[64:128, :], ps_k[0:64, :],
                                     sink[64:128, :])
                rtk = mp.tile([128, 512], BF16, tag="m", name="rtk")
                nc.vector.tensor_add(rtk[:, :], m1k[:, :], m2k[:, :])
                nc.vector.tensor_mul(k_sb[:, :], rtk[:, :], rk[:, :])

                q4 = q_sb.rearrange("p (h b s) -> p h b s", h=HQC, b=B)

                def outproj(b, act_copy=False):
                    _outproj(b, o_sb, wot, act_copy=act_copy)

                # ---- attention ----
                for bi, b in enumerate(border):
                    ncache = nch[b]
                    tail = lens[b] - (ncache - 1) * 128 if ncache > 0 else 0
                    cis = list(range(ncache + 1))
                    groups = [cis[i:i + 2] for i in range(0, len(cis), 2)]
                    ngr = len(groups)
                    # [0:512] = unnormalized o, [512:1024] = prob sums
                    ps_os = psO.tile([128, 1024], F32, tag="po",
                                     name=f"pos{b}")

                    def kchunk(ci, b=b, ncache=ncache):
                        if ci == ncache:
                            return k_sb[:, b * S:(b + 1) * S]
                        return kTt[:, offs[b] + ci * 128:offs[b] + (ci + 1) * 128]

                    def vchunk(ci, b=b, ncache=ncache):
                        if ci == ncache:
                            return vt[:, b * S:(b + 1) * S]
                        return vCt[:, offs[b] + ci * 128:offs[b] + (ci + 1) * 128]

                    pending = []
                    # eager binary reduction forest of prob slices; level-3
                    # roots (8 slices) are matmul'd into the sum as they form
                    forest = []
                    nslices = ncache + 1
                    # outproj halves of the previous batch spread across this
                    # batch's groups to plug the Exp-vs-PE rate gap
                    op_gis = ([max(1, round((j + 1) * ngr / 5.0))
                               for j in range(4)] if bi > 0 else [])
                    # dry-run the flush schedule to learn how many slices are
                    # left pending at the end (they become direct sum roots,
                    # keeping DVE off the batch-end critical path)
                    pend_n = 0
                    for gi_, grp_ in enumerate(groups):
                        pend_n += 1
                        pd_ = 1 if (bi == B - 1 and gi_ >= ngr - 3) else 2
                        pend_n = min(pend_n, max(pd_, 1))
                    n_tail = sum(len(g) for g in groups[ngr - pend_n:])
                    pushed = nslices - n_tail
                    total_roots = (pushed // 8 + bin(pushed % 8).count("1")
                                   + n_tail)
                    sst = {'open': False, 'roots_left': total_roots}

                    def emit_root(ap, ps_os=ps_os):
                        st = not sst['open']
                        sst['open'] = True
                        sst['roots_left'] -= 1
                        nc.tensor.matmul(ps_os[:, 512:1024], allones, ap,
                                         start=st, stop=sst['roots_left'] == 0)

                    def push_prob(pr):
                        forest.append((0, pr))
                        while (len(forest) >= 2
                               and forest[-1][0] == forest[-2][0]):
                            l2, a2 = forest.pop()
                            l1, a1 = forest.pop()
                            t = pp2.tile([128, 512], BF16, tag="pp2")
                            nc.vector.tensor_add(t[:, :], a1, a2)
                            if l1 + 1 == 3:
                                emit_root(t[:, :])
                            else:
                                forest.append((l1 + 1, t[:, :]))

                    def drain_forest():
                        for _, ap in forest:
                            emit_root(ap)
                        forest.clear()

                    def flush(gi_, prob_, width_, ps_os=ps_os, ngr=ngr,
                              groups=groups, push=True):
                        first = gi_ == 0
                        last = gi_ == ngr - 1
                        nk = width_ // 512
                        for k in range(nk):
                            ci = groups[gi_][k]
                            pr = prob_[:, k * 512:(k + 1) * 512]
                            st = first and k == 0
                            sp = last and k == nk - 1
                            nc.tensor.matmul(ps_os[:, 0:512], vchunk(ci), pr,
                                             start=st, stop=sp)
                            if push:
                                push_prob(pr)

                    mci = ncache - 1 if (ncache > 0 and tail < 128) else -1
                    for gi, grp in enumerate(groups):
                        width = 512 * len(grp)
                        ps_s = psS.tile([128, 1024], F32, tag="ps",
                                        name=f"s{b}_{gi}")
                        for k, ci in enumerate(grp):
                            nc.tensor.matmul(ps_s[:, k * 512:(k + 1) * 512],
                                             kchunk(ci), q4[:, :, b, :],
                                             start=True, stop=True)
                        prob = probp.tile([128, 1024], BF16, tag="prob")
                        if mci in grp:
                            for k, ci in enumerate(grp):
                                if ci == mci:
                                    nc.scalar.activation(
                                        prob[:, k * 512:(k + 1) * 512],
                                        ps_s[:, k * 512:(k + 1) * 512],
                                        AF.Exp, bias=maskb[b], scale=1.0)
                                else:
                                    nc.scalar.activation(
                                        prob[:, k * 512:(k + 1) * 512],
                                        ps_s[:, k * 512:(k + 1) * 512],
                                        AF.Exp)
                        else:
                            nc.scalar.activation(prob[:, 0:width],
                                                 ps_s[:, 0:width], AF.Exp)
                        pending.append((gi, prob, width))
                        if bi == B - 1 and gi >= ngr - 3:
                            pdepth = 1
                        else:
                            pdepth = 2
                        while len(pending) > pdepth:
                            flush(*pending.pop(0))
                        if gi in op_gis:
                            for hj in range(4):
                                if op_gis[hj] == gi:
                                    _outproj_half(border[bi - 1], o_sb, wot,
                                                  hj)
                    # close the sums first (pending slices as direct roots),
                    # so the reciprocal overlaps the tail PV matmuls
                    for (gi_, prob_, width_) in pending:
                        for k in range(width_ // 512):
                            emit_root(prob_[:, k * 512:(k + 1) * 512])
                    drain_forest()
                    recb = recp.tile([128, 512], F32, tag="rec")
                    nc.vector.reciprocal(recb[:, :], ps_os[:, 512:1024])
                    while pending:
                        flush(*pending.pop(0), push=False)
                    nc.vector.tensor_mul(o_sb[:, b * 512:(b + 1) * 512],
                                         ps_os[:, 0:512], recb[:, :])
                outproj(border[-1], act_copy=True)

            if reps == 1:
                body(0)
            else:
                with tc.For_i(0, reps, 1,
                              hint_engines=(mybir.EngineType.PE,
                                            mybir.EngineType.Activation,
                                            mybir.EngineType.Pool,
                                            mybir.EngineType.DVE,
                                            mybir.EngineType.SP)) as it:
                    body(it)

    nc.compile()
    return nc


def _get_nc(lens, pads, offs, total, reps=1, phases=3):
    key = (tuple(lens), total, reps)
    if key not in _CACHE:
        _CACHE[key] = _build_nc(lens, pads, offs, total, reps)
    return _CACHE[key]


def kernel(x, Wq, Wk, Wv, Wo, q_norm_w, k_norm_w, k_cache, v_cache,
           block_table, cache_seqlens):
    from concourse.bass_utils import run_bass_kernel_spmd

    in_maps, lens, pads, offs, total = _prep_host(
        x, Wq, Wk, Wv, Wo, q_norm_w, k_norm_w, k_cache, v_cache,
        block_table, cache_seqlens)
    nc = _get_nc(lens, pads, offs, total, reps=1)
    res = run_bass_kernel_spmd(nc, in_maps, core_ids=list(range(N_CORES)))
    partials = np.stack([np.asarray(r["out"], np.float32)
                         for r in res.results], 0)
    out = np.sum(partials, axis=0, dtype=np.float64).astype(np.float32)
    return out.reshape(B, S, HID)
